# revision 34
# baseline (speedup 1.0000x reference)
"""CenterLoss kernel for Trainium2 (8 NeuronCores, SPMD data-parallel).

Math: for pixel p with feature x_p (256-ch), label l_p, centers C[19,256]:
    mean dist = 2 - (2/B) * S,   S = sum_p (x_p . cn_{l_p}) / ||x_p||,
with cn = C/||C|| row-wise (||xn||^2 == ||cn||^2 == 1 up to rounding).

Device plan (fp8 x + fp8 weighted-one-hot; per-pixel quantization noise
averages out over 65536 px; measured rel err ~4e-6):
  - x ships as fp8e4m3 in 7 tiles (256..2048 px) alternating across the
    two HWDGE queues. Sizes ascend then descend: the first two tiles are
    small so compute starts ~2us earlier, the last tiles are small so
    the post-DMA tail is short. The normalized centers ride in front of
    tile 0's transfer and the weighted one-hot in front of tile 1's, so
    there are only 7 DMA issues total (each costs ~700ns of issue time,
    and each HWDGE queue only has 4 completion semaphores — more
    transfers per queue stall on semaphore recycling).
  - labels ship folded with the per-pixel norm weight: ohw[32q+k, col]
    = alpha/||x_p|| for pixel p in strip q of its tile (fp8, 256KB).
  - per tile t:
      dots [128, W/4] PSUM : 8 plain-fp8 MMs on 4 col-strips (strips run
                             concurrently on PE column groups), stationary
                             = normalized centers.
      affine_mul_reduce    : one DVE op reads dots straight from PSUM,
                             multiplies by the ohw column block and
                             row-reduces into asum[:, t] (fp32); a tiny
                             tensor_add folds it into a running [128,1]
                             sum so the finish needs no [1,N] reduce.
  - finish: ones-matmul collapses the running sum [128,1] -> [1,1] PSUM,
    and gpsimd reg_load/reg_save writes the 4 bytes to DRAM directly (a
    DMA'd output pays 1.5-2.5us of completion-semaphore latency before
    the final barrier can pass; the SWDGE ring drain costs ~1.6us).
  - host: loss = 2 - (2/B) * (sum of 8 core scalars) / alpha.

Known dead ends (measured on this stack): tensor_tensor_reduce wedges the
device (NRT_EXEC_UNIT_UNRECOVERABLE) in every dtype/space combination;
the PE never ramps past ~1.2 GHz regardless of HAM warm-up, so warm-up
matmuls only delay real work; DMA completion semaphores trail the data
by 0.5-2us per transfer (size-independent), so finer tiling beyond ~7
transfers loses more to semaphore lag + issue cost than it gains.
More dead ends: fp8 DoubleRow (one matmul for the 256-ch contraction)
fails walrus's s3d3_mm_valid_dst_partition ISA check for any non-zero
output partition base, so it cannot combine with the 4-strip partition
stacking (and unstacked output would idle 3/4 of the DVE lanes);
reg_load from PSUM, gpsimd tensor_copy, and gpsimd
scalar_tensor_tensor all fail walrus codegen (Pool-engine DVE-class ops
are not lowerable), so the select-reduce tail cannot be split across
engines.
"""

import sys

import numpy as np

if "/opt/trn_rl_repo" not in sys.path:
    sys.path.insert(0, "/opt/trn_rl_repo")

import concourse.bacc as bacc
import concourse.tile as tile
from concourse import mybir
from concourse.bass_utils import run_bass_kernel_spmd

N_CORES = 8
C = 256
NCLS = 19
N_IMG, H, W = 4, 128, 128
PIX_TOTAL = N_IMG * H * W            # 65536
PIX_PER_CORE = PIX_TOTAL // N_CORES  # 8192
# pixels per tile (sum = 8192). Strip width w/4 must stay <= 512 so each
# matmul's [32, w/4] f32 PSUM slab fits one 2KB bank. Tiles alternate
# queues (even -> sync, odd -> scalar); byte totals per queue balance to
# ~1.15MB each including the centers (tile 0) and ohw (tile 1) prefixes.
TILE_PX = [256, 512, 2048, 2048, 1792, 1024, 512]
N_TILES = len(TILE_PX)
OH_F = PIX_PER_CORE // 4             # 2048 ohw columns
F32 = mybir.dt.float32
I32 = mybir.dt.int32
FP8 = mybir.dt.float8e4

ALPHA = 16.0  # keeps ohw ~1 and the selected products ~N(0,1) in fp8 range


def build_nc():
    """Build the per-core Bass program (same program on all 8 cores)."""
    nc = bacc.Bacc(None, target_bir_lowering=False, debug=False)
    x_d = []
    for t in range(N_TILES):
        cols = 2 * TILE_PX[t]
        if t == 0:
            cols += 64       # ctn [2,32] prefix
        elif t == 1:
            cols += OH_F     # ohw prefix
        x_d.append(
            nc.dram_tensor(f"x{t}", [128, cols], FP8, kind="ExternalInput")
        )
    out_d = nc.dram_tensor("out", [1, 1], F32, kind="ExternalOutput")

    with tile.TileContext(nc) as tc:
        with (
            tc.tile_pool(name="consts", bufs=1) as consts,
            tc.tile_pool(name="xin", bufs=1) as xin,
            tc.tile_pool(name="work", bufs=2) as work,
            tc.tile_pool(name="fin", bufs=1) as finp,
            tc.tile_pool(name="dots", bufs=3, space="PSUM") as dotsp,
            tc.tile_pool(name="fps", bufs=1, space="PSUM") as fpsp,
        ):
            # ---- input DMAs (both HWDGE queues, tiles alternating) ----
            xts = []
            for t in range(N_TILES):
                xt_t = xin.tile(
                    [128, x_d[t].shape[1]], FP8, tag=f"xt{t}", name=f"xt{t}"
                )
                eng = nc.sync if t % 2 == 0 else nc.scalar
                eng.dma_start(out=xt_t[:], in_=x_d[t][:])
                xts.append(xt_t)
            ct = xts[0]   # cols 0:32 = half-0 centers, 32:64 = half-1
            oh = xts[1]   # cols 0:OH_F = weighted one-hot

            # ---- constants ----
            ones128 = consts.tile([128, 1], F32, tag="ones128")
            nc.vector.memset(ones128[:], 1.0)

            asum = finp.tile([128, N_TILES], F32, tag="asum")
            run = finp.tile([128, 1], F32, tag="run")
            res = finp.tile([1, 1], F32, tag="res")

            # ---- main loop ----
            for t in range(N_TILES):
                xt_t = xts[t]
                w = TILE_PX[t]
                qw = w // 4
                base = {0: 64, 1: OH_F}.get(t, 0)  # skip ct/ohw prefix
                dots4 = dotsp.tile(
                    [128, 512], F32, tag="dots4", name="dots4"
                )[:, 0:qw]
                for h in range(2):
                    for q in range(4):
                        # full 32-col stationary: cols 19-31 are zeros, so
                        # strip rows 19-31 are written clean
                        nc.tensor.matmul(
                            dots4[32 * q : 32 * q + 32, :],
                            ct[:, 32 * h : 32 * h + 32],
                            xt_t[:, base + h * w + q * qw :
                                 base + h * w + (q + 1) * qw],
                            start=(h == 0),
                            stop=(h == 1),
                            tile_position=(0, 32 * q),
                            # CoreSim's zero-region group check ignores the
                            # partition base, so concurrent col-strips look
                            # like conflicts; HW handles them fine.
                            skip_group_check=True,
                        )
                # one DVE op: select+weight (mult by ohw) and row-reduce.
                # prodsel is write-only scratch; the reduction is the result.
                off = sum(TILE_PX[:t]) // 4
                prodsel = work.tile(
                    [128, 512], FP8, tag="prodsel", name="prodsel"
                )[:, 0:qw]
                nc.vector.affine_mul_reduce(
                    out=prodsel[:],
                    accum_out=asum[:, t : t + 1],
                    in0=dots4[:],
                    in1=oh[:, off : off + qw],
                    scale=1.0,
                    bias=0.0,
                )
                # fold into a running [128,1] sum (tiny Vector op, hides
                # mid-stream) so the finish needs no [1,N] reduce
                if t == 0:
                    nc.vector.tensor_copy(run[:], asum[:, 0:1])
                else:
                    nc.vector.tensor_add(
                        out=run[:], in0=run[:], in1=asum[:, t : t + 1]
                    )

            # ---- collapse partitions: [128,1] -> [1,1] ----
            fin4 = fpsp.tile([1, 1], F32, tag="fin4")
            nc.tensor.matmul(
                fin4[:], ones128[:], run[:], start=True, stop=True
            )
            nc.vector.tensor_copy(res[:], fin4[:])
            # 4-byte result via engine registers: skips the DMA completion
            # semaphore (which trickles in 1.5-2.5us after the data lands).
            # reg_load straight from PSUM skips the PSUM->SBUF copy hop.
            with nc.gpsimd.register("rres") as rres:
                nc.gpsimd.reg_load(rres, res[0:1, 0:1].bitcast(I32))
                nc.gpsimd.reg_save(out_d[0:1, 0:1].bitcast(I32), rres)

    nc.compile()
    return nc


def shard_inputs(x, centers, labels):
    """Full inputs -> list of 8 per-core input maps (fp8 on-device)."""
    import ml_dtypes

    FP8NP = ml_dtypes.float8_e4m3fn
    x = np.asarray(x, dtype=np.float32)
    centers = np.asarray(centers, dtype=np.float32)
    labels = np.asarray(labels)

    x8 = x.astype(FP8NP)
    #   [n, 2(ch-half), 128(ch), 2(core-half), 8192(px)]
    xr = x8.reshape(N_IMG, 2, 128, 2, PIX_PER_CORE)
    labr = labels.reshape(N_IMG, 2, PIX_PER_CORE).astype(np.int64)

    # per-pixel weight from the QUANTIZED x (cancels fp8 scale noise a bit)
    xq = x8.astype(np.float32).reshape(N_IMG, C, 2, PIX_PER_CORE)
    nrm = np.sqrt(np.maximum((xq * xq).sum(axis=1), 1e-24))  # [n, 2, 8192]
    wgt = (ALPHA / nrm).astype(np.float32)

    # ct [128, 64]: cols 32h+k = cn[k, 128h + p] for partition p
    cn = centers / np.maximum(
        np.linalg.norm(centers, axis=1, keepdims=True), 1e-12
    )
    ct = np.zeros((128, 64), dtype=FP8NP)
    cre = cn.astype(FP8NP).reshape(NCLS, 2, 128)
    for h in range(2):
        ct[:, 32 * h : 32 * h + NCLS] = cre[:, h, :].T

    # per-pixel tile index / strip / column for the variable-width tiles
    bounds = np.cumsum([0] + TILE_PX)
    px = np.arange(PIX_PER_CORE)
    tidx = np.searchsorted(bounds, px, side="right") - 1
    within = px - bounds[tidx]
    qws = np.array(TILE_PX) // 4
    offs = np.cumsum([0] + list(qws[:-1]))
    rows_q = 32 * (within // qws[tidx])
    cols = offs[tidx] + within % qws[tidx]

    in_maps = []
    for core in range(N_CORES):
        n, j = core // 2, core % 2
        xcore = xr[n, :, :, j, :]  # [2(h), 128, 8192]
        lab = labr[n, j]
        oh = np.zeros((128, OH_F), dtype=np.float32)
        oh[rows_q + lab, cols] = wgt[n, j]
        m = {}
        for t in range(N_TILES):
            lo, hi = bounds[t], bounds[t + 1]
            xt = np.ascontiguousarray(
                xcore[:, :, lo:hi].transpose(1, 0, 2)
            ).reshape(128, 2 * TILE_PX[t])  # [128, 2*W_t], h-major
            if t == 0:
                xt = np.concatenate([ct, xt], axis=1)
            elif t == 1:
                xt = np.concatenate([oh.astype(FP8NP), xt], axis=1)
            m[f"x{t}"] = xt
        in_maps.append(m)
    return in_maps


_NC_CACHE = {}


def _ensure_ntff_hook():
    """Register the axon NTFF profile hook if the optional antenv.axon_hooks
    module is absent from this image (bass_utils hard-imports it when
    trace=True)."""
    try:
        from antenv.axon_hooks import get_axon_ntff_profile_hook  # noqa: F401

        return
    except ImportError:
        pass
    import types

    import antenv

    mod = types.ModuleType("antenv.axon_hooks")
    state = {"hook": None}
    mod.set_axon_ntff_profile_hook = lambda h: state.__setitem__("hook", h)
    mod.get_axon_ntff_profile_hook = lambda: state["hook"]
    sys.modules["antenv.axon_hooks"] = mod
    antenv.axon_hooks = mod
    try:
        from trn_agent_boot.trn_boot import _ntff_profile_via_ctypes

        mod.set_axon_ntff_profile_hook(
            _ntff_profile_via_ctypes("/opt/axon/libaxon_pjrt.so")
        )
    except Exception:
        pass


def kernel(x, centers, labels, _profile=False):
    in_maps = shard_inputs(x, centers, labels)
    if _profile:
        _ensure_ntff_hook()
    if "nc" not in _NC_CACHE:
        _NC_CACHE["nc"] = build_nc()
    nc = _NC_CACHE["nc"]
    res = run_bass_kernel_spmd(
        nc, in_maps, list(range(N_CORES)), trace=bool(_profile)
    )
    s = 0.0
    for r in res.results:
        s += float(np.asarray(r["out"], dtype=np.float64).reshape(()))
    val = np.array(np.float32(2.0 - 2.0 * s / (ALPHA * PIX_TOTAL)))
    if _profile:
        return val, res
    return val
